# revision 22
# baseline (speedup 1.0000x reference)
"""Trainium2 Bass kernel: per-cluster PCA geometry features (segment reduce).

Problem: data [4194304, 6] f32, clusts [32768, 128] int — per cluster of 128
voxels compute: center (mean of xyz), normalized covariance B = A/lmax,
principal axis v0 scaled by dirwt = 1 - lmid/lmax with a sign fix, size.

v2 design (cost-model driven):
- 32768 clusters sharded over 8 cores (4096 each); cluster c = p*32 + s
  (partition p, segment s). All plane data in bf16.
- Dual layout per coordinate: voxel-major [V, C_LOC] for PE column sums
  (phase-1 moments), cluster-major *segment-minor* [P, V, S] so per-cluster
  scalar broadcasts keep a packed (stride-1) last dim -> DVE 2x mode.
- Phase 1: squares on ACT, cross products on GPSIMD, sums via 288 tiny
  PE matmuls into one PSUM tile.
- Eigensolve: batched small-tile ops on [128, K, 32] tiles (trig method),
  eigenvector via row-cross-products on a doubled-rows tile so the three
  candidate cross products are affine-sliced batched ops.
- Phase 2 (per segment-half, pipelined): xc = X - c, r2 = |xc|^2 via ACT
  squares + GPSIMD adds, x0' = xc . e (unnormalized eigvec, avoids a sqrt
  on the critical path), h = r2 - x0'^2/|e|^2, np0 = ACT sqrt of clamped h,
  sign criterion sc = sum_v x0'*np0 via packed tree-reduction.
"""
import numpy as np
from contextlib import ExitStack

import concourse.bass as bass
import concourse.bacc as bacc
import concourse.tile as tile
from concourse import mybir
from concourse.bass_utils import run_bass_kernel_spmd

N_CLUSTS = 32768
CLUST_SIZE = 128
N_CORES = 8
C_LOC = N_CLUSTS // N_CORES   # 4096 clusters per core
P = 128                       # SBUF partitions
NSEG = C_LOC // P             # 32 clusters (segments) per partition
V = CLUST_SIZE                # 128 voxels per cluster
HALF = NSEG // 2

F32 = mybir.dt.float32
BF16 = mybir.dt.bfloat16
AF = mybir.ActivationFunctionType
OP = mybir.AluOpType
AX = mybir.AxisListType

_CACHED = {}
PROFILE = False          # set by test.py to capture an NTFF trace
LAST_RESULT = None       # BassKernelResults of the last run when PROFILE
INPUT_SHAPES = {
    "x": (P, V * NSEG), "y": (P, V * NSEG), "z": (P, V * NSEG),
    "xt": (V, C_LOC), "yt": (V, C_LOC), "zt": (V, C_LOC),
}
INPUT_DTYPE = "bfloat16"


def build_nc():
    nc = bacc.Bacc()
    # cluster-major segment-minor: row p holds [v, s] (v outer, s inner)
    x_d = nc.dram_tensor("x", [P, V * NSEG], BF16, kind="ExternalInput").ap()
    y_d = nc.dram_tensor("y", [P, V * NSEG], BF16, kind="ExternalInput").ap()
    z_d = nc.dram_tensor("z", [P, V * NSEG], BF16, kind="ExternalInput").ap()
    # voxel-major: column s*128+p holds cluster p*32+s
    xt_d = nc.dram_tensor("xt", [V, C_LOC], BF16, kind="ExternalInput").ap()
    yt_d = nc.dram_tensor("yt", [V, C_LOC], BF16, kind="ExternalInput").ap()
    zt_d = nc.dram_tensor("zt", [V, C_LOC], BF16, kind="ExternalInput").ap()
    feats_d = nc.dram_tensor("feats", [C_LOC, 16], F32, kind="ExternalOutput").ap()

    with tile.TileContext(nc) as tc, ExitStack() as ctx:
        pool = ctx.enter_context(tc.tile_pool(name="main", bufs=1))
        sp = ctx.enter_context(tc.tile_pool(name="scratch", bufs=6))
        spc = ctx.enter_context(tc.tile_pool(name="scratchc", bufs=4))
        spt = ctx.enter_context(tc.tile_pool(name="scratcht", bufs=3))
        pp = ctx.enter_context(tc.tile_pool(name="psum", bufs=1, space="PSUM"))

        ones = pool.tile([P, 1], BF16, tag="ones")
        nc.gpsimd.memset(ones[:], 1.0)
        warm = pool.tile([P, 1], F32, tag="warm")
        nc.gpsimd.memset(warm[:], 1.0)
        nc.scalar.activation(warm[:], warm[:], AF.Sqrt)

        # ---------------- input DMAs ----------------
        Xt = pool.tile([P, C_LOC], BF16, tag="Xt")
        Yt = pool.tile([P, C_LOC], BF16, tag="Yt")
        Zt = pool.tile([P, C_LOC], BF16, tag="Zt")
        X = pool.tile([P, V, NSEG], BF16, tag="X")
        Y = pool.tile([P, V, NSEG], BF16, tag="Y")
        Z = pool.tile([P, V, NSEG], BF16, tag="Z")
        nc.sync.dma_start(Yt[:], yt_d)
        nc.sync.dma_start(Zt[:], zt_d)
        nc.sync.dma_start(Xt[:], xt_d)
        nc.sync.dma_start(X[:], x_d.rearrange("p (v s) -> p v s", s=NSEG))
        nc.sync.dma_start(Y[:], y_d.rearrange("p (v s) -> p v s", s=NSEG))
        nc.sync.dma_start(Z[:], z_d.rearrange("p (v s) -> p v s", s=NSEG))

        # ---------------- phase 1: moments via PE column sums ----------------
        psum = pp.tile([P, 9, NSEG], F32, tag="psums")

        def pe_colsums(plane_t, k):
            for j in range(NSEG):
                nc.tensor.matmul(
                    out=psum[:, k, j : j + 1],
                    lhsT=plane_t[:, j * P : (j + 1) * P],
                    rhs=ones[:, 0:1],
                    start=True,
                    stop=True,
                )

        pe_colsums(Yt, 1)
        sq2 = sp.tile([P, C_LOC], BF16, tag="vplane", name="sq2")
        nc.scalar.activation(sq2[:], Yt[:], AF.Square)
        pe_colsums(sq2, 4)
        pe_colsums(Zt, 2)
        sq3 = sp.tile([P, C_LOC], BF16, tag="vplane", name="sq3")
        nc.scalar.activation(sq3[:], Zt[:], AF.Square)
        pe_colsums(sq3, 5)
        pe_colsums(Xt, 0)
        mom = pool.tile([P, 9, NSEG], F32, tag="mom")
        nc.vector.tensor_copy(mom[:, 0:3], psum[:, 0:3])
        inv_s = 1.0 / V
        c3 = pool.tile([P, 3, NSEG], F32, tag="c3")
        nc.vector.tensor_scalar(out=c3[:], in0=mom[:, 0:3], scalar1=inv_s,
                                scalar2=None, op0=OP.mult)
        cb = pool.tile([P, 3, NSEG], BF16, tag="cb")
        nc.vector.tensor_copy(cb[:], c3[:])
        pr3 = sp.tile([P, C_LOC], BF16, tag="vplane", name="pr3")
        nc.gpsimd.tensor_tensor(pr3[:], Yt[:], Zt[:], OP.mult)
        pe_colsums(pr3, 8)
        sq1 = sp.tile([P, C_LOC], BF16, tag="vplane", name="sq1")
        nc.vector.tensor_tensor(sq1[:], Xt[:], Xt[:], OP.mult)
        pe_colsums(sq1, 3)
        pr1 = sp.tile([P, C_LOC], BF16, tag="vplane", name="pr1")
        nc.gpsimd.tensor_tensor(pr1[:], Xt[:], Yt[:], OP.mult)
        pe_colsums(pr1, 6)
        pr2 = sp.tile([P, C_LOC], BF16, tag="vplane", name="pr2")
        nc.vector.tensor_tensor(pr2[:], Xt[:], Zt[:], OP.mult)
        pe_colsums(pr2, 7)

        # second moments: rows 3-8 = [Mxx Myy Mzz Mxy Mxz Myz]
        nc.vector.tensor_copy(mom[:, 3:9], psum[:, 3:9])

        def bcast_v(t, lo, hi, sl, n=1):
            # [P, K, NSEG] rows lo:hi, segments sl -> [P, n, V, W] broadcast
            w = sl.stop - sl.start
            if n == 1:
                return t[:, lo:hi, sl].broadcast_to([P, V, w]) if hi - lo == 1 \
                    else None
            return None

        # a = M - S*c ; amat rows: [axx, ayy, azz, axy, axz, ayz]
        amat = pool.tile([P, 6, NSEG], F32, tag="amat")
        t6 = pool.tile([P, 6, NSEG], F32, tag="t6")
        nc.vector.tensor_tensor(t6[:, 0:3], mom[:, 0:3], c3[:], OP.mult)
        nc.vector.tensor_tensor(
            t6[:, 3:5], mom[:, 0:1].broadcast_to([P, 2, NSEG]),
            c3[:, 1:3], OP.mult)
        nc.vector.tensor_tensor(t6[:, 5:6], mom[:, 1:2], c3[:, 2:3], OP.mult)
        nc.vector.tensor_tensor(amat[:], mom[:, 3:9], t6[:], OP.subtract)

        # ---- phase 2 state ------------------------------------------------
        planes = [X, Y, Z]
        r2h = [pool.tile([P, V, HALF], BF16, tag=f"r2h{h}", name=f"r2h{h}")
               for h in range(2)]
        x0h = [pool.tile([P, V, HALF], BF16, tag=f"x0h{h}", name=f"x0h{h}")
               for h in range(2)]
        sqc = [[spc.tile([P, V, HALF], BF16, tag="sqc", name=f"sqc{i}{h}")
                for h in range(2)] for i in range(3)]

        def cbc(t, row, h, w=HALF):
            # [P, K, NSEG] row -> [P, V, w] broadcast over voxels
            return t[:, row:row + 1, h * HALF:h * HALF + w].broadcast_to(
                [P, V, w])

        def xc_i(i, eng=None):
            # center one coordinate (both halves)
            eng = eng or (nc.vector if i == 0 else nc.gpsimd)
            pl = planes[i]
            for h in range(2):
                sl = slice(h * HALF, (h + 1) * HALF)
                eng.tensor_tensor(pl[:, :, sl], pl[:, :, sl],
                                  cbc(cb, i, h), OP.subtract)

        def sqc_i(i):
            # square one centered coordinate (both halves) on ACT
            pl = planes[i]
            for h in range(2):
                sl = slice(h * HALF, (h + 1) * HALF)
                nc.scalar.activation(sqc[i][h][:], pl[:, :, sl], AF.Square)

        # a = M - S*c ; amat rows: [axx, ayy, azz, axy, axz, ayz]
        amat = pool.tile([P, 6, NSEG], F32, tag="amat")
        t6 = pool.tile([P, 6, NSEG], F32, tag="t6")
        nc.vector.tensor_tensor(t6[:, 0:3], mom[:, 0:3], c3[:], OP.mult)
        nc.vector.tensor_tensor(
            t6[:, 3:5], mom[:, 0:1].broadcast_to([P, 2, NSEG]),
            c3[:, 1:3], OP.mult)
        nc.vector.tensor_tensor(t6[:, 5:6], mom[:, 1:2], c3[:, 2:3], OP.mult)
        nc.vector.tensor_tensor(amat[:], mom[:, 3:9], t6[:], OP.subtract)

        # ---------------- eigensolve (batched [P, K, NSEG] f32) -------------
        q = pool.tile([P, 1, NSEG], F32, tag="q")
        t1a = pool.tile([P, 1, NSEG], F32, tag="t1a")
        t1b = pool.tile([P, 1, NSEG], F32, tag="t1b")
        nc.vector.tensor_tensor(t1a[:], amat[:, 0:1], amat[:, 1:2], OP.add)
        nc.vector.tensor_tensor(t1a[:], t1a[:], amat[:, 2:3], OP.add)
        nc.vector.tensor_scalar(out=q[:], in0=t1a[:], scalar1=1.0 / 3.0,
                                scalar2=None, op0=OP.mult)
        # ba rows: [b11, b22, b33, axy, axz, ayz]  (traceless part)
        ba = pool.tile([P, 6, NSEG], F32, tag="ba")
        nc.vector.tensor_tensor(
            ba[:, 0:3], amat[:, 0:3], q[:].broadcast_to([P, 3, NSEG]),
            OP.subtract)
        nc.vector.tensor_copy(ba[:, 3:6], amat[:, 3:6])

        # p2 = sum(bd^2) + 2*sum(off^2) ; p = sqrt(p2/6); invp = 1/p
        sq6 = pool.tile([P, 6, NSEG], F32, tag="sq6")
        nc.vector.tensor_tensor(sq6[:], ba[:], ba[:], OP.mult)
        w3t = pool.tile([P, 3, NSEG], F32, tag="w3t")
        nc.vector.scalar_tensor_tensor(out=w3t[:], in0=sq6[:, 3:6], scalar=2.0,
                                       in1=sq6[:, 0:3], op0=OP.mult, op1=OP.add)
        p2s = pool.tile([P, 1, NSEG], F32, tag="p2s")
        nc.vector.tensor_tensor(t1a[:], w3t[:, 0:1], w3t[:, 1:2], OP.add)
        nc.vector.tensor_tensor(p2s[:], t1a[:], w3t[:, 2:3], OP.add)
        p_t = pool.tile([P, 1, NSEG], F32, tag="p_t")
        nc.scalar.activation(p_t[:], p2s[:], AF.Sqrt, scale=1.0 / 6.0)
        invp = pool.tile([P, 1, NSEG], F32, tag="invp")
        nc.vector.reciprocal(invp[:], p_t[:])

        # center + square x (DVE was busy with the eigen chain until now)
        xc_i(0)
        sqc_i(0)
        xc_i(1)

        # det of the raw traceless matrix ba on DVE (overlaps the ACT sqrt
        # for p); r = det(ba) * invp^3 / 2, clamped to [-1, 1].
        # minors vs row1: m1 = b22*b33 - b23^2 ; m2 = b12*b33 - b23*b13 ;
        # m3 = b12*b23 - b22*b13 ; det = b11*m1 - b12*m2 + b13*m3
        r_t = pool.tile([P, 1, NSEG], F32, tag="r_t")
        det_a = pool.tile([P, 3, NSEG], F32, tag="det_a")
        det_b = pool.tile([P, 3, NSEG], F32, tag="det_b")
        nc.vector.tensor_tensor(det_a[:, 0:1], ba[:, 1:2], ba[:, 2:3],
                                OP.mult)
        nc.vector.tensor_tensor(
            det_a[:, 1:3], ba[:, 3:4].broadcast_to([P, 2, NSEG]),
            ba[:, 2:6:3], OP.mult)
        nc.vector.tensor_tensor(det_b[:, 0:1], ba[:, 5:6], ba[:, 5:6],
                                OP.mult)
        nc.vector.tensor_tensor(det_b[:, 1:2], ba[:, 5:6], ba[:, 4:5],
                                OP.mult)
        nc.vector.tensor_tensor(det_b[:, 2:3], ba[:, 1:2], ba[:, 4:5],
                                OP.mult)
        nc.vector.tensor_tensor(det_a[:], det_a[:], det_b[:], OP.subtract)
        nc.vector.tensor_tensor(det_b[:, 0:1], ba[:, 0:1], det_a[:, 0:1],
                                OP.mult)
        nc.vector.tensor_tensor(det_b[:, 1:3], ba[:, 3:5], det_a[:, 1:3],
                                OP.mult)
        nc.vector.tensor_tensor(t1b[:], det_b[:, 0:1], det_b[:, 1:2],
                                OP.subtract)
        nc.vector.tensor_tensor(t1b[:], t1b[:], det_b[:, 2:3], OP.add)
        # r = det * invp^3 / 2, clamped
        nc.vector.tensor_tensor(t1a[:], invp[:], invp[:], OP.mult)
        nc.vector.tensor_tensor(t1a[:], t1a[:], invp[:], OP.mult)
        nc.vector.tensor_tensor(t1b[:], t1b[:], t1a[:], OP.mult)
        nc.vector.tensor_scalar(out=r_t[:], in0=t1b[:], scalar1=0.5,
                                scalar2=1.0, op0=OP.mult, op1=OP.min)
        nc.vector.tensor_scalar(out=r_t[:], in0=r_t[:], scalar1=-1.0,
                                scalar2=None, op0=OP.max)

        # square y on ACT; center z on GPSIMD (z DMA lands about now)
        sqc_i(1)
        xc_i(2)

        # f = cos(acos(r)/3): largest root of 4f^3 - 3f = r.
        # Chebyshev deg-8 seed (err 2.2e-2) + 2 Newton steps -> 5.5e-3 max
        # (only near r=-1, where the top eigenpair degenerates and dirwt -> 0).
        SEED = [0.8649279174994734, 0.15532929881670984, -0.0021054445875550026,
                0.14553392157332898, -0.3069890177054866, -0.3028838631742596,
                0.5725439670593226, 0.24058719928428143, -0.3682048402442527]
        f_t = pool.tile([P, 1, NSEG], F32, tag="f_t")
        nc.vector.tensor_scalar(out=f_t[:], in0=r_t[:], scalar1=0.0,
                                scalar2=SEED[8], op0=OP.mult, op1=OP.add)
        for k in range(8, 0, -1):
            # f <- (f + a_k) * r, then finally + a_0
            nc.vector.scalar_tensor_tensor(
                out=f_t[:], in0=f_t[:], scalar=0.0 if k == 8 else SEED[k],
                in1=r_t[:], op0=OP.add, op1=OP.mult)
        nc.vector.tensor_scalar(out=f_t[:], in0=f_t[:], scalar1=SEED[0],
                                scalar2=None, op0=OP.add)
        nc.vector.tensor_scalar(out=f_t[:], in0=f_t[:], scalar1=0.5,
                                scalar2=1.0, op0=OP.max, op1=OP.min)
        f2 = pool.tile([P, 1, NSEG], F32, tag="f2")
        for _ in range(1):
            nc.vector.tensor_tensor(f2[:], f_t[:], f_t[:], OP.mult)
            nc.vector.tensor_scalar(out=t1a[:], in0=f2[:], scalar1=12.0,
                                    scalar2=-3.0, op0=OP.mult, op1=OP.add)
            nc.vector.tensor_scalar(out=t1a[:], in0=t1a[:], scalar1=1e-3,
                                    scalar2=None, op0=OP.max)
            nc.vector.reciprocal(t1a[:], t1a[:])
            nc.vector.tensor_scalar(out=t1b[:], in0=f2[:], scalar1=4.0,
                                    scalar2=-3.0, op0=OP.mult, op1=OP.add)
            nc.vector.tensor_tensor(t1b[:], t1b[:], f_t[:], OP.mult)
            nc.vector.tensor_tensor(t1b[:], t1b[:], r_t[:], OP.subtract)
            nc.vector.tensor_tensor(t1b[:], t1b[:], t1a[:], OP.mult)
            nc.vector.tensor_tensor(f_t[:], f_t[:], t1b[:], OP.subtract)
            nc.vector.tensor_scalar(out=f_t[:], in0=f_t[:], scalar1=0.5,
                                    scalar2=1.0, op0=OP.max, op1=OP.min)

        # w3 = q + 2*p*f ; w2 = q + (sqrt(2*p2*(1-f^2)) - 2*p*f)/2
        # (w2 from the deflated quadratic t^2 + t3 t + t3^2 - p2/2 = 0)
        w3 = pool.tile([P, 1, NSEG], F32, tag="w3")
        w2 = pool.tile([P, 1, NSEG], F32, tag="w2")
        m1 = pool.tile([P, 1, NSEG], F32, tag="m1")
        nc.vector.tensor_tensor(m1[:], p_t[:], f_t[:], OP.mult)   # p*f
        nc.vector.scalar_tensor_tensor(out=w3[:], in0=m1[:], scalar=2.0,
                                       in1=q[:], op0=OP.mult, op1=OP.add)
        nc.vector.tensor_tensor(f2[:], f_t[:], f_t[:], OP.mult)
        nc.vector.tensor_scalar(out=t1b[:], in0=f2[:], scalar1=-1.0,
                                scalar2=1.0, op0=OP.mult, op1=OP.add)
        nc.vector.scalar_tensor_tensor(out=t1b[:], in0=p2s[:], scalar=2.0,
                                       in1=t1b[:], op0=OP.mult, op1=OP.mult)
        nc.scalar.activation(t1a[:], t1b[:], AF.Sqrt)   # sqrt(disc)
        nc.vector.scalar_tensor_tensor(out=t1b[:], in0=t1a[:], scalar=0.5,
                                       in1=m1[:], op0=OP.mult, op1=OP.subtract)
        nc.vector.tensor_tensor(w2[:], t1b[:], q[:], OP.add)

        invw3 = pool.tile([P, 1, NSEG], F32, tag="invw3")
        nc.vector.reciprocal(invw3[:], w3[:])
        dirwt = pool.tile([P, 1, NSEG], F32, tag="dirwt")
        nc.vector.tensor_tensor(dirwt[:], w2[:], invw3[:], OP.mult)
        nc.vector.tensor_scalar(out=dirwt[:], in0=dirwt[:], scalar1=-1.0,
                                scalar2=1.0, op0=OP.mult, op1=OP.add)

        # square z on ACT; r2 = sum of squares on GPSIMD
        sqc_i(2)
        for h, eng in ((0, nc.vector), (1, nc.gpsimd)):
            eng.tensor_tensor(r2h[h][:], sqc[0][h][:], sqc[1][h][:], OP.add)
            eng.tensor_tensor(r2h[h][:], r2h[h][:], sqc[2][h][:], OP.add)

        # ---- eigenvector: batched row cross products -------------------
        # rows of (A - w3 I): r1 = (d1, axy, axz), r2 = (axy, d2, ayz),
        # r3 = (axz, ayz, d3); D = per-vector doubled rows.
        rows = pool.tile([P, 9, NSEG], F32, tag="rows")
        nc.vector.tensor_tensor(
            rows[:, 0:9:4], amat[:, 0:3],
            w3[:].broadcast_to([P, 3, NSEG]), OP.subtract)
        nc.vector.tensor_copy(rows[:, 1:2], amat[:, 3:4])   # axy
        nc.vector.tensor_copy(rows[:, 2:3], amat[:, 4:5])   # axz
        nc.vector.tensor_copy(rows[:, 3:4], amat[:, 3:4])   # axy
        nc.vector.tensor_copy(rows[:, 5:6], amat[:, 5:6])   # ayz
        nc.vector.tensor_copy(rows[:, 6:7], amat[:, 4:5])   # axz
        nc.vector.tensor_copy(rows[:, 7:8], amat[:, 5:6])   # ayz
        D = pool.tile([P, 3, 2, 3, NSEG], F32, tag="D")
        rows_g = rows[:].rearrange("p (g c) s -> p g c s", g=3)
        nc.vector.tensor_copy(D[:, :, 0], rows_g)
        nc.vector.tensor_copy(D[:, :, 1], rows_g)
        Dg = D[:].rearrange("p g r c s -> p g (r c) s")  # [P, 3, 6, NSEG]

        # VN rows: cand_a(0:3), na(3), cand_b(4:7), nb(7), cand_c(8:11), nc(11)
        VN = pool.tile([P, 3, 4, NSEG], F32, tag="VN")
        ta = pool.tile([P, 2, 3, NSEG], F32, tag="ta")
        tb = pool.tile([P, 2, 3, NSEG], F32, tag="tb")
        r1s1 = Dg[:, 0:1, 1:4].broadcast_to([P, 2, 3, NSEG])
        r1s2 = Dg[:, 0:1, 2:5].broadcast_to([P, 2, 3, NSEG])
        nc.vector.tensor_tensor(ta[:], r1s1, Dg[:, 1:3, 2:5], OP.mult)
        nc.vector.tensor_tensor(tb[:], r1s2, Dg[:, 1:3, 1:4], OP.mult)
        nc.vector.tensor_tensor(ta[:], ta[:], tb[:], OP.subtract)
        nc.vector.tensor_copy(VN[:, 0:2, 0:3], ta[:])
        nc.vector.tensor_tensor(tb[:, 0], Dg[:, 1, 1:4], Dg[:, 2, 2:5], OP.mult)
        nc.vector.tensor_tensor(tb[:, 1], Dg[:, 1, 2:5], Dg[:, 2, 1:4], OP.mult)
        nc.vector.tensor_tensor(VN[:, 2, 0:3], tb[:, 0], tb[:, 1], OP.subtract)

        # norms into VN[:, :, 3]
        sq9 = pool.tile([P, 3, 3, NSEG], F32, tag="sq9")
        nc.vector.tensor_tensor(sq9[:], VN[:, :, 0:3], VN[:, :, 0:3], OP.mult)
        nc.vector.tensor_tensor(ta[:, 0], sq9[:, :, 0], sq9[:, :, 1], OP.add)
        nc.vector.tensor_tensor(VN[:, :, 3], ta[:, 0], sq9[:, :, 2], OP.add)

        # select the largest-norm candidate (two rounds, batched 4-row select)
        m4 = pool.tile([P, 4, NSEG], mybir.dt.uint8, tag="mask4")
        best = pool.tile([P, 4, NSEG], F32, tag="best")
        nc.vector.tensor_tensor(
            m4[:], VN[:, 1, 3:4].broadcast_to([P, 4, NSEG]),
            VN[:, 0, 3:4].broadcast_to([P, 4, NSEG]), OP.is_gt)
        nc.vector.select(best[:], m4[:], VN[:, 1], VN[:, 0])
        nc.vector.tensor_tensor(
            m4[:], VN[:, 2, 3:4].broadcast_to([P, 4, NSEG]),
            best[:, 3:4].broadcast_to([P, 4, NSEG]), OP.is_gt)
        nc.vector.select(best[:], m4[:], VN[:, 2], best[:])

        # rsqn = 1/|e| ; normalized eigvec in bf16
        rn = pool.tile([P, 1, NSEG], F32, tag="rn")
        nc.vector.reciprocal(rn[:], best[:, 3:4])
        rsqn = pool.tile([P, 1, NSEG], F32, tag="rsqn")
        nc.scalar.activation(rsqn[:], rn[:], AF.Sqrt)
        en = pool.tile([P, 3, NSEG], F32, tag="en")
        nc.vector.tensor_tensor(
            en[:], best[:, 0:3], rsqn[:].broadcast_to([P, 3, NSEG]), OP.mult)
        eb = pool.tile([P, 3, NSEG], BF16, tag="eb")
        nc.vector.tensor_copy(eb[:], en[:])

        # ---------------- phase 2 tail: per segment-half --------------------
        # h0's chain runs on DVE, h1's front half on GPSIMD, so the two
        # halves pipeline across engines; np0 sqrts on ACT.
        sc = pool.tile([P, 2, HALF], F32, tag="sc")
        uhs, u2s, xqs = [], [], []
        for h in range(2):
            uhs.append(spt.tile([P, V, HALF], BF16, tag="uh", name=f"uh{h}"))
            u2s.append(spt.tile([P, V, HALF], BF16, tag="uh2", name=f"uh2{h}"))
            xqs.append(spt.tile([P, V, HALF], BF16, tag="x0sq",
                                name=f"x0sq{h}"))
        for h, eng in ((0, nc.vector), (1, nc.gpsimd)):
            sl = slice(h * HALF, (h + 1) * HALF)
            # x0 = xc . e (normalized)
            eng.tensor_tensor(x0h[h][:], X[:, :, sl], cbc(eb, 0, h), OP.mult)
            eng.tensor_tensor(uhs[h][:], Y[:, :, sl], cbc(eb, 1, h), OP.mult)
            eng.tensor_tensor(u2s[h][:], Z[:, :, sl], cbc(eb, 2, h), OP.mult)
            eng.tensor_tensor(x0h[h][:], x0h[h][:], uhs[h][:], OP.add)
            eng.tensor_tensor(x0h[h][:], x0h[h][:], u2s[h][:], OP.add)
        for h in range(2):
            # h = r2 - x0^2 ; np0 = sqrt(max(h, 0)) ; pr = x0*np0 ; tree-sum
            nc.vector.tensor_tensor(xqs[h][:], x0h[h][:], x0h[h][:], OP.mult)
            nc.vector.tensor_tensor(r2h[h][:], r2h[h][:], xqs[h][:],
                                    OP.subtract)
            nc.vector.tensor_scalar(out=r2h[h][:], in0=r2h[h][:], scalar1=0.0,
                                    scalar2=None, op0=OP.max)
            nc.scalar.activation(r2h[h][:], r2h[h][:], AF.Sqrt)
            nc.vector.tensor_tensor(x0h[h][:], x0h[h][:], r2h[h][:], OP.mult)
            pr = x0h[h]
            w = V
            while w > 8:
                w //= 2
                nc.vector.tensor_tensor(pr[:, 0:w], pr[:, 0:w],
                                        pr[:, w:2 * w], OP.add)
            nc.vector.tensor_reduce(
                sc[:, h], pr[:, 0:8].rearrange("p v s -> p s v"),
                axis=AX.X, op=OP.add)

        # ---------------- finalize: feats [P, NSEG, 16] ----------------
        feats = pool.tile([P, NSEG, 16], F32, tag="feats")
        sgn = pool.tile([P, 1, NSEG], F32, tag="sgn")
        nc.scalar.activation(sgn[:, 0], sc[:].rearrange("p h s -> p (h s)"),
                             AF.Sign)
        fac = pool.tile([P, 1, NSEG], F32, tag="fac")
        nc.vector.tensor_tensor(fac[:], dirwt[:], rsqn[:], OP.mult)
        nc.vector.tensor_tensor(fac[:], fac[:], sgn[:], OP.mult)

        nc.gpsimd.tensor_copy(
            feats[:, :, 0:3].rearrange("p s k -> p k s"), c3[:])
        bb = pool.tile([P, 6, NSEG], F32, tag="bb")
        nc.vector.tensor_tensor(
            bb[:], amat[:], invw3[:].broadcast_to([P, 6, NSEG]), OP.mult)
        for col, row in ((3, 0), (4, 3), (5, 4), (6, 3), (7, 1), (8, 5),
                         (9, 4), (10, 5), (11, 2)):
            nc.gpsimd.tensor_copy(feats[:, :, col], bb[:, row])
        v0t = pool.tile([P, 3, NSEG], F32, tag="v0t")
        nc.vector.tensor_tensor(
            v0t[:], best[:, 0:3], fac[:].broadcast_to([P, 3, NSEG]), OP.mult)
        nc.gpsimd.tensor_copy(
            feats[:, :, 12:15].rearrange("p s k -> p k s"), v0t[:])
        size_t = pool.tile([P, NSEG], F32, tag="size_t")
        nc.gpsimd.memset(size_t[:], float(V))
        nc.gpsimd.tensor_copy(feats[:, :, 15], size_t[:])

        nc.sync.dma_start(feats_d.rearrange("(p s) k -> p s k", p=P), feats[:])

    if not nc.is_finalized():
        nc.finalize()
    return nc


def kernel(data: np.ndarray, clusts: np.ndarray) -> np.ndarray:
    import ml_dtypes
    BF = ml_dtypes.bfloat16

    data = np.asarray(data, dtype=np.float32)
    clusts_np = np.asarray(clusts)
    C, S = clusts_np.shape
    assert (C, S) == (N_CLUSTS, CLUST_SIZE), (C, S)

    vox = data[:, 1:4]
    g = vox[clusts_np.reshape(-1).astype(np.int64)].reshape(C, S, 3)
    g = g.astype(BF)

    if "nc" not in _CACHED:
        _CACHED["nc"] = build_nc()
    nc = _CACHED["nc"]

    in_maps = []
    for c in range(N_CORES):
        sl = slice(c * C_LOC, (c + 1) * C_LOC)
        gc = g[sl]  # [C_LOC, S, 3]
        m = {}
        for i, n in enumerate("xyz"):
            a4 = gc[:, :, i].reshape(P, NSEG, V)
            # cluster-major segment-minor [P, V, NSEG]
            m[n] = np.ascontiguousarray(a4.transpose(0, 2, 1)).reshape(
                P, V * NSEG)
            # voxel-major [V, C_LOC], column s*128+p = cluster p*32+s
            m[n + "t"] = np.ascontiguousarray(
                a4.transpose(2, 1, 0).reshape(V, C_LOC))
        in_maps.append(m)

    kw = {}
    if PROFILE:
        kw = dict(trace=True)
    res = run_bass_kernel_spmd(nc, in_maps, list(range(N_CORES)), **kw)
    if PROFILE:
        global LAST_RESULT
        LAST_RESULT = res
    out = np.concatenate([res.results[c]["feats"] for c in range(N_CORES)],
                         axis=0)
    return out.astype(np.float32)


# revision 25
# speedup vs baseline: 1.0114x; 1.0114x over previous
"""Trainium2 Bass kernel: per-cluster PCA geometry features (segment reduce).

Problem: data [4194304, 6] f32, clusts [32768, 128] int — per cluster of 128
voxels compute: center (mean of xyz), normalized covariance B = A/lmax,
principal axis v0 scaled by dirwt = 1 - lmid/lmax with a sign fix, size.

v2 design (cost-model driven):
- 32768 clusters sharded over 8 cores (4096 each); cluster c = p*32 + s
  (partition p, segment s). All plane data in bf16.
- Dual layout per coordinate: voxel-major [V, C_LOC] for PE column sums
  (phase-1 moments), cluster-major *segment-minor* [P, V, S] so per-cluster
  scalar broadcasts keep a packed (stride-1) last dim -> DVE 2x mode.
- Phase 1: squares on ACT, cross products on GPSIMD, sums via 288 tiny
  PE matmuls into one PSUM tile.
- Eigensolve: batched small-tile ops on [128, K, 32] tiles (trig method),
  eigenvector via row-cross-products on a doubled-rows tile so the three
  candidate cross products are affine-sliced batched ops.
- Phase 2 (per segment-half, pipelined): xc = X - c, r2 = |xc|^2 via ACT
  squares + GPSIMD adds, x0' = xc . e (unnormalized eigvec, avoids a sqrt
  on the critical path), h = r2 - x0'^2/|e|^2, np0 = ACT sqrt of clamped h,
  sign criterion sc = sum_v x0'*np0 via packed tree-reduction.
"""
import numpy as np
from contextlib import ExitStack

import concourse.bass as bass
import concourse.bacc as bacc
import concourse.tile as tile
from concourse import mybir
from concourse.bass_utils import run_bass_kernel_spmd

N_CLUSTS = 32768
CLUST_SIZE = 128
N_CORES = 8
C_LOC = N_CLUSTS // N_CORES   # 4096 clusters per core
P = 128                       # SBUF partitions
NSEG = C_LOC // P             # 32 clusters (segments) per partition
V = CLUST_SIZE                # 128 voxels per cluster
HALF = NSEG // 2

F32 = mybir.dt.float32
BF16 = mybir.dt.bfloat16
AF = mybir.ActivationFunctionType
OP = mybir.AluOpType
AX = mybir.AxisListType

_CACHED = {}
PROFILE = False          # set by test.py to capture an NTFF trace
LAST_RESULT = None       # BassKernelResults of the last run when PROFILE
INPUT_SHAPES = {
    "x": (P, V * NSEG), "y": (P, V * NSEG), "z": (P, V * NSEG),
    "xt": (V, C_LOC), "yt": (V, C_LOC), "zt": (V, C_LOC),
}
INPUT_DTYPE = "bfloat16"


def build_nc():
    nc = bacc.Bacc()
    # cluster-major segment-minor: row p holds [v, s] (v outer, s inner)
    x_d = nc.dram_tensor("x", [P, V * NSEG], BF16, kind="ExternalInput").ap()
    y_d = nc.dram_tensor("y", [P, V * NSEG], BF16, kind="ExternalInput").ap()
    z_d = nc.dram_tensor("z", [P, V * NSEG], BF16, kind="ExternalInput").ap()
    # voxel-major: column s*128+p holds cluster p*32+s
    xt_d = nc.dram_tensor("xt", [V, C_LOC], BF16, kind="ExternalInput").ap()
    yt_d = nc.dram_tensor("yt", [V, C_LOC], BF16, kind="ExternalInput").ap()
    zt_d = nc.dram_tensor("zt", [V, C_LOC], BF16, kind="ExternalInput").ap()
    feats_d = nc.dram_tensor("feats", [C_LOC, 16], F32, kind="ExternalOutput").ap()

    with tile.TileContext(nc) as tc, ExitStack() as ctx:
        pool = ctx.enter_context(tc.tile_pool(name="main", bufs=1))
        sp = ctx.enter_context(tc.tile_pool(name="scratch", bufs=6))
        spc = ctx.enter_context(tc.tile_pool(name="scratchc", bufs=4))
        spt = ctx.enter_context(tc.tile_pool(name="scratcht", bufs=3))
        pp = ctx.enter_context(tc.tile_pool(name="psum", bufs=1, space="PSUM"))

        ones = pool.tile([P, 1], BF16, tag="ones")
        nc.gpsimd.memset(ones[:], 1.0)
        warm = pool.tile([P, 1], F32, tag="warm")
        nc.gpsimd.memset(warm[:], 1.0)
        nc.scalar.activation(warm[:], warm[:], AF.Sqrt)

        # ---------------- input DMAs ----------------
        Xt = pool.tile([P, C_LOC], BF16, tag="Xt")
        Yt = pool.tile([P, C_LOC], BF16, tag="Yt")
        Zt = pool.tile([P, C_LOC], BF16, tag="Zt")
        X = pool.tile([P, V, NSEG], BF16, tag="X")
        Y = pool.tile([P, V, NSEG], BF16, tag="Y")
        Z = pool.tile([P, V, NSEG], BF16, tag="Z")
        nc.sync.dma_start(Xt[:], xt_d)
        nc.sync.dma_start(Yt[:], yt_d)
        nc.sync.dma_start(Zt[:], zt_d)
        nc.sync.dma_start(X[:], x_d.rearrange("p (v s) -> p v s", s=NSEG))
        nc.sync.dma_start(Y[:], y_d.rearrange("p (v s) -> p v s", s=NSEG))
        nc.sync.dma_start(Z[:], z_d.rearrange("p (v s) -> p v s", s=NSEG))

        # ---------------- phase 1: moments via PE column sums ----------------
        psum = pp.tile([P, 9, NSEG], F32, tag="psums")

        def pe_colsums(plane_t, k):
            for j in range(NSEG):
                nc.tensor.matmul(
                    out=psum[:, k, j : j + 1],
                    lhsT=plane_t[:, j * P : (j + 1) * P],
                    rhs=ones[:, 0:1],
                    start=True,
                    stop=True,
                )

        pe_colsums(Xt, 0)
        pe_colsums(Yt, 1)
        pe_colsums(Zt, 2)
        mom = pool.tile([P, 9, NSEG], F32, tag="mom")
        nc.vector.tensor_copy(mom[:, 0:3], psum[:, 0:3])
        inv_s = 1.0 / V
        c3 = pool.tile([P, 3, NSEG], F32, tag="c3")
        nc.vector.tensor_scalar(out=c3[:], in0=mom[:, 0:3], scalar1=inv_s,
                                scalar2=None, op0=OP.mult)
        cb = pool.tile([P, 3, NSEG], BF16, tag="cb")
        nc.vector.tensor_copy(cb[:], c3[:])
        sq1 = sp.tile([P, C_LOC], BF16, tag="vplane", name="sq1")
        nc.scalar.activation(sq1[:], Xt[:], AF.Square)
        pe_colsums(sq1, 3)
        sq2 = sp.tile([P, C_LOC], BF16, tag="vplane", name="sq2")
        nc.scalar.activation(sq2[:], Yt[:], AF.Square)
        pe_colsums(sq2, 4)
        pr1 = sp.tile([P, C_LOC], BF16, tag="vplane", name="pr1")
        nc.gpsimd.tensor_tensor(pr1[:], Xt[:], Yt[:], OP.mult)
        pe_colsums(pr1, 6)
        pr2 = sp.tile([P, C_LOC], BF16, tag="vplane", name="pr2")
        nc.vector.tensor_tensor(pr2[:], Xt[:], Zt[:], OP.mult)
        pe_colsums(pr2, 7)
        # last plane's square split across ACT and DVE halves
        sq3 = sp.tile([P, C_LOC], BF16, tag="vplane", name="sq3")
        nc.scalar.activation(sq3[:, 0:C_LOC // 2], Zt[:, 0:C_LOC // 2],
                             AF.Square)
        nc.vector.tensor_tensor(sq3[:, C_LOC // 2:], Zt[:, C_LOC // 2:],
                                Zt[:, C_LOC // 2:], OP.mult)
        pe_colsums(sq3, 5)
        pr3 = sp.tile([P, C_LOC], BF16, tag="vplane", name="pr3")
        nc.gpsimd.tensor_tensor(pr3[:], Yt[:], Zt[:], OP.mult)
        pe_colsums(pr3, 8)

        # second moments: rows 3-8 = [Mxx Myy Mzz Mxy Mxz Myz]
        nc.vector.tensor_copy(mom[:, 3:9], psum[:, 3:9])

        def bcast_v(t, lo, hi, sl, n=1):
            # [P, K, NSEG] rows lo:hi, segments sl -> [P, n, V, W] broadcast
            w = sl.stop - sl.start
            if n == 1:
                return t[:, lo:hi, sl].broadcast_to([P, V, w]) if hi - lo == 1 \
                    else None
            return None

        # a = M - S*c ; amat rows: [axx, ayy, azz, axy, axz, ayz]
        amat = pool.tile([P, 6, NSEG], F32, tag="amat")
        t6 = pool.tile([P, 6, NSEG], F32, tag="t6")
        nc.vector.tensor_tensor(t6[:, 0:3], mom[:, 0:3], c3[:], OP.mult)
        nc.vector.tensor_tensor(
            t6[:, 3:5], mom[:, 0:1].broadcast_to([P, 2, NSEG]),
            c3[:, 1:3], OP.mult)
        nc.vector.tensor_tensor(t6[:, 5:6], mom[:, 1:2], c3[:, 2:3], OP.mult)
        nc.vector.tensor_tensor(amat[:], mom[:, 3:9], t6[:], OP.subtract)

        # ---- phase 2 state ------------------------------------------------
        planes = [X, Y, Z]
        r2h = [pool.tile([P, V, HALF], BF16, tag=f"r2h{h}", name=f"r2h{h}")
               for h in range(2)]
        x0h = [pool.tile([P, V, HALF], BF16, tag=f"x0h{h}", name=f"x0h{h}")
               for h in range(2)]
        sqc = [[spc.tile([P, V, HALF], BF16, tag="sqc", name=f"sqc{i}{h}")
                for h in range(2)] for i in range(3)]

        def cbc(t, row, h, w=HALF):
            # [P, K, NSEG] row -> [P, V, w] broadcast over voxels
            return t[:, row:row + 1, h * HALF:h * HALF + w].broadcast_to(
                [P, V, w])

        def xc_i(i, eng=None):
            # center one coordinate (both halves)
            eng = eng or (nc.vector if i == 0 else nc.gpsimd)
            pl = planes[i]
            for h in range(2):
                sl = slice(h * HALF, (h + 1) * HALF)
                eng.tensor_tensor(pl[:, :, sl], pl[:, :, sl],
                                  cbc(cb, i, h), OP.subtract)

        def sqc_i(i):
            # square one centered coordinate (both halves) on ACT
            pl = planes[i]
            for h in range(2):
                sl = slice(h * HALF, (h + 1) * HALF)
                nc.scalar.activation(sqc[i][h][:], pl[:, :, sl], AF.Square)

        # a = M - S*c ; amat rows: [axx, ayy, azz, axy, axz, ayz]
        amat = pool.tile([P, 6, NSEG], F32, tag="amat")
        t6 = pool.tile([P, 6, NSEG], F32, tag="t6")
        nc.vector.tensor_tensor(t6[:, 0:3], mom[:, 0:3], c3[:], OP.mult)
        nc.vector.tensor_tensor(
            t6[:, 3:5], mom[:, 0:1].broadcast_to([P, 2, NSEG]),
            c3[:, 1:3], OP.mult)
        nc.vector.tensor_tensor(t6[:, 5:6], mom[:, 1:2], c3[:, 2:3], OP.mult)
        nc.vector.tensor_tensor(amat[:], mom[:, 3:9], t6[:], OP.subtract)

        # ---------------- eigensolve (batched [P, K, NSEG] f32) -------------
        q = pool.tile([P, 1, NSEG], F32, tag="q")
        t1a = pool.tile([P, 1, NSEG], F32, tag="t1a")
        t1b = pool.tile([P, 1, NSEG], F32, tag="t1b")
        nc.vector.tensor_tensor(t1a[:], amat[:, 0:1], amat[:, 1:2], OP.add)
        nc.vector.tensor_tensor(t1a[:], t1a[:], amat[:, 2:3], OP.add)
        nc.vector.tensor_scalar(out=q[:], in0=t1a[:], scalar1=1.0 / 3.0,
                                scalar2=None, op0=OP.mult)
        # ba rows: [b11, b22, b33, axy, axz, ayz]  (traceless part)
        ba = pool.tile([P, 6, NSEG], F32, tag="ba")
        nc.vector.tensor_tensor(
            ba[:, 0:3], amat[:, 0:3], q[:].broadcast_to([P, 3, NSEG]),
            OP.subtract)
        nc.vector.tensor_copy(ba[:, 3:6], amat[:, 3:6])

        # p2 = sum(bd^2) + 2*sum(off^2) ; p = sqrt(p2/6); invp = 1/p
        sq6 = pool.tile([P, 6, NSEG], F32, tag="sq6")
        nc.vector.tensor_tensor(sq6[:], ba[:], ba[:], OP.mult)
        w3t = pool.tile([P, 3, NSEG], F32, tag="w3t")
        nc.vector.scalar_tensor_tensor(out=w3t[:], in0=sq6[:, 3:6], scalar=2.0,
                                       in1=sq6[:, 0:3], op0=OP.mult, op1=OP.add)
        p2s = pool.tile([P, 1, NSEG], F32, tag="p2s")
        nc.vector.tensor_tensor(t1a[:], w3t[:, 0:1], w3t[:, 1:2], OP.add)
        nc.vector.tensor_tensor(p2s[:], t1a[:], w3t[:, 2:3], OP.add)
        p_t = pool.tile([P, 1, NSEG], F32, tag="p_t")
        nc.scalar.activation(p_t[:], p2s[:], AF.Sqrt, scale=1.0 / 6.0)
        invp = pool.tile([P, 1, NSEG], F32, tag="invp")
        nc.vector.reciprocal(invp[:], p_t[:])

        # center + square x (DVE was busy with the eigen chain until now)
        xc_i(0)
        sqc_i(0)
        xc_i(1)

        # det of the raw traceless matrix ba on DVE (overlaps the ACT sqrt
        # for p); r = det(ba) * invp^3 / 2, clamped to [-1, 1].
        # minors vs row1: m1 = b22*b33 - b23^2 ; m2 = b12*b33 - b23*b13 ;
        # m3 = b12*b23 - b22*b13 ; det = b11*m1 - b12*m2 + b13*m3
        r_t = pool.tile([P, 1, NSEG], F32, tag="r_t")
        det_a = pool.tile([P, 3, NSEG], F32, tag="det_a")
        det_b = pool.tile([P, 3, NSEG], F32, tag="det_b")
        nc.vector.tensor_tensor(det_a[:, 0:1], ba[:, 1:2], ba[:, 2:3],
                                OP.mult)
        nc.vector.tensor_tensor(
            det_a[:, 1:3], ba[:, 3:4].broadcast_to([P, 2, NSEG]),
            ba[:, 2:6:3], OP.mult)
        nc.vector.tensor_tensor(det_b[:, 0:1], ba[:, 5:6], ba[:, 5:6],
                                OP.mult)
        nc.vector.tensor_tensor(det_b[:, 1:2], ba[:, 5:6], ba[:, 4:5],
                                OP.mult)
        nc.vector.tensor_tensor(det_b[:, 2:3], ba[:, 1:2], ba[:, 4:5],
                                OP.mult)
        nc.vector.tensor_tensor(det_a[:], det_a[:], det_b[:], OP.subtract)
        nc.vector.tensor_tensor(det_b[:, 0:1], ba[:, 0:1], det_a[:, 0:1],
                                OP.mult)
        nc.vector.tensor_tensor(det_b[:, 1:3], ba[:, 3:5], det_a[:, 1:3],
                                OP.mult)
        nc.vector.tensor_tensor(t1b[:], det_b[:, 0:1], det_b[:, 1:2],
                                OP.subtract)
        nc.vector.tensor_tensor(t1b[:], t1b[:], det_b[:, 2:3], OP.add)
        # r = det * invp^3 / 2, clamped
        nc.vector.tensor_tensor(t1a[:], invp[:], invp[:], OP.mult)
        nc.vector.tensor_tensor(t1a[:], t1a[:], invp[:], OP.mult)
        nc.vector.tensor_tensor(t1b[:], t1b[:], t1a[:], OP.mult)
        nc.vector.tensor_scalar(out=r_t[:], in0=t1b[:], scalar1=0.5,
                                scalar2=1.0, op0=OP.mult, op1=OP.min)
        nc.vector.tensor_scalar(out=r_t[:], in0=r_t[:], scalar1=-1.0,
                                scalar2=None, op0=OP.max)

        # square y on ACT; center z on GPSIMD (z DMA lands about now)
        sqc_i(1)
        xc_i(2)

        # f = cos(acos(r)/3): largest root of 4f^3 - 3f = r.
        # Chebyshev deg-8 seed (err 2.2e-2) + 2 Newton steps -> 5.5e-3 max
        # (only near r=-1, where the top eigenpair degenerates and dirwt -> 0).
        SEED = [0.8649279174994734, 0.15532929881670984, -0.0021054445875550026,
                0.14553392157332898, -0.3069890177054866, -0.3028838631742596,
                0.5725439670593226, 0.24058719928428143, -0.3682048402442527]
        f_t = pool.tile([P, 1, NSEG], F32, tag="f_t")
        nc.vector.tensor_scalar(out=f_t[:], in0=r_t[:], scalar1=0.0,
                                scalar2=SEED[8], op0=OP.mult, op1=OP.add)
        for k in range(8, 0, -1):
            # f <- (f + a_k) * r, then finally + a_0
            nc.vector.scalar_tensor_tensor(
                out=f_t[:], in0=f_t[:], scalar=0.0 if k == 8 else SEED[k],
                in1=r_t[:], op0=OP.add, op1=OP.mult)
        nc.vector.tensor_scalar(out=f_t[:], in0=f_t[:], scalar1=SEED[0],
                                scalar2=None, op0=OP.add)
        nc.vector.tensor_scalar(out=f_t[:], in0=f_t[:], scalar1=0.5,
                                scalar2=1.0, op0=OP.max, op1=OP.min)
        f2 = pool.tile([P, 1, NSEG], F32, tag="f2")
        for _ in range(1):
            nc.vector.tensor_tensor(f2[:], f_t[:], f_t[:], OP.mult)
            nc.vector.tensor_scalar(out=t1a[:], in0=f2[:], scalar1=12.0,
                                    scalar2=-3.0, op0=OP.mult, op1=OP.add)
            nc.vector.tensor_scalar(out=t1a[:], in0=t1a[:], scalar1=1e-3,
                                    scalar2=None, op0=OP.max)
            nc.vector.reciprocal(t1a[:], t1a[:])
            nc.vector.tensor_scalar(out=t1b[:], in0=f2[:], scalar1=4.0,
                                    scalar2=-3.0, op0=OP.mult, op1=OP.add)
            nc.vector.tensor_tensor(t1b[:], t1b[:], f_t[:], OP.mult)
            nc.vector.tensor_tensor(t1b[:], t1b[:], r_t[:], OP.subtract)
            nc.vector.tensor_tensor(t1b[:], t1b[:], t1a[:], OP.mult)
            nc.vector.tensor_tensor(f_t[:], f_t[:], t1b[:], OP.subtract)
            nc.vector.tensor_scalar(out=f_t[:], in0=f_t[:], scalar1=0.5,
                                    scalar2=1.0, op0=OP.max, op1=OP.min)

        # w3 = q + 2*p*f ; w2 = q + (sqrt(2*p2*(1-f^2)) - 2*p*f)/2
        # (w2 from the deflated quadratic t^2 + t3 t + t3^2 - p2/2 = 0)
        w3 = pool.tile([P, 1, NSEG], F32, tag="w3")
        w2 = pool.tile([P, 1, NSEG], F32, tag="w2")
        m1 = pool.tile([P, 1, NSEG], F32, tag="m1")
        nc.vector.tensor_tensor(m1[:], p_t[:], f_t[:], OP.mult)   # p*f
        nc.vector.scalar_tensor_tensor(out=w3[:], in0=m1[:], scalar=2.0,
                                       in1=q[:], op0=OP.mult, op1=OP.add)
        nc.vector.tensor_tensor(f2[:], f_t[:], f_t[:], OP.mult)
        nc.vector.tensor_scalar(out=t1b[:], in0=f2[:], scalar1=-1.0,
                                scalar2=1.0, op0=OP.mult, op1=OP.add)
        nc.vector.scalar_tensor_tensor(out=t1b[:], in0=p2s[:], scalar=2.0,
                                       in1=t1b[:], op0=OP.mult, op1=OP.mult)
        nc.scalar.activation(t1a[:], t1b[:], AF.Sqrt)   # sqrt(disc)
        nc.vector.scalar_tensor_tensor(out=t1b[:], in0=t1a[:], scalar=0.5,
                                       in1=m1[:], op0=OP.mult, op1=OP.subtract)
        nc.vector.tensor_tensor(w2[:], t1b[:], q[:], OP.add)

        invw3 = pool.tile([P, 1, NSEG], F32, tag="invw3")
        nc.vector.reciprocal(invw3[:], w3[:])
        dirwt = pool.tile([P, 1, NSEG], F32, tag="dirwt")
        nc.vector.tensor_tensor(dirwt[:], w2[:], invw3[:], OP.mult)
        nc.vector.tensor_scalar(out=dirwt[:], in0=dirwt[:], scalar1=-1.0,
                                scalar2=1.0, op0=OP.mult, op1=OP.add)

        # ---- eigenvector: batched row cross products -------------------
        # rows of (A - w3 I): r1 = (d1, axy, axz), r2 = (axy, d2, ayz),
        # r3 = (axz, ayz, d3); D = per-vector doubled rows.
        rows = pool.tile([P, 9, NSEG], F32, tag="rows")
        nc.vector.tensor_tensor(
            rows[:, 0:9:4], amat[:, 0:3],
            w3[:].broadcast_to([P, 3, NSEG]), OP.subtract)
        nc.vector.tensor_copy(rows[:, 1:2], amat[:, 3:4])   # axy
        nc.vector.tensor_copy(rows[:, 2:3], amat[:, 4:5])   # axz
        nc.vector.tensor_copy(rows[:, 3:4], amat[:, 3:4])   # axy
        nc.vector.tensor_copy(rows[:, 5:6], amat[:, 5:6])   # ayz
        nc.vector.tensor_copy(rows[:, 6:7], amat[:, 4:5])   # axz
        nc.vector.tensor_copy(rows[:, 7:8], amat[:, 5:6])   # ayz
        D = pool.tile([P, 3, 2, 3, NSEG], F32, tag="D")
        rows_g = rows[:].rearrange("p (g c) s -> p g c s", g=3)
        nc.vector.tensor_copy(D[:, :, 0], rows_g)
        nc.vector.tensor_copy(D[:, :, 1], rows_g)
        Dg = D[:].rearrange("p g r c s -> p g (r c) s")  # [P, 3, 6, NSEG]

        # VN rows: cand_a(0:3), na(3), cand_b(4:7), nb(7), cand_c(8:11), nc(11)
        VN = pool.tile([P, 3, 4, NSEG], F32, tag="VN")
        ta = pool.tile([P, 2, 3, NSEG], F32, tag="ta")
        tb = pool.tile([P, 2, 3, NSEG], F32, tag="tb")
        r1s1 = Dg[:, 0:1, 1:4].broadcast_to([P, 2, 3, NSEG])
        r1s2 = Dg[:, 0:1, 2:5].broadcast_to([P, 2, 3, NSEG])
        nc.vector.tensor_tensor(ta[:], r1s1, Dg[:, 1:3, 2:5], OP.mult)
        nc.vector.tensor_tensor(tb[:], r1s2, Dg[:, 1:3, 1:4], OP.mult)
        nc.vector.tensor_tensor(ta[:], ta[:], tb[:], OP.subtract)
        nc.vector.tensor_copy(VN[:, 0:2, 0:3], ta[:])
        nc.vector.tensor_tensor(tb[:, 0], Dg[:, 1, 1:4], Dg[:, 2, 2:5], OP.mult)
        nc.vector.tensor_tensor(tb[:, 1], Dg[:, 1, 2:5], Dg[:, 2, 1:4], OP.mult)
        nc.vector.tensor_tensor(VN[:, 2, 0:3], tb[:, 0], tb[:, 1], OP.subtract)

        # norms into VN[:, :, 3]
        sq9 = pool.tile([P, 3, 3, NSEG], F32, tag="sq9")
        nc.vector.tensor_tensor(sq9[:], VN[:, :, 0:3], VN[:, :, 0:3], OP.mult)
        nc.vector.tensor_tensor(ta[:, 0], sq9[:, :, 0], sq9[:, :, 1], OP.add)
        nc.vector.tensor_tensor(VN[:, :, 3], ta[:, 0], sq9[:, :, 2], OP.add)

        # select the largest-norm candidate (two rounds, batched 4-row select)
        m4 = pool.tile([P, 4, NSEG], mybir.dt.uint8, tag="mask4")
        best = pool.tile([P, 4, NSEG], F32, tag="best")
        nc.vector.tensor_tensor(
            m4[:], VN[:, 1, 3:4].broadcast_to([P, 4, NSEG]),
            VN[:, 0, 3:4].broadcast_to([P, 4, NSEG]), OP.is_gt)
        nc.vector.select(best[:], m4[:], VN[:, 1], VN[:, 0])
        nc.vector.tensor_tensor(
            m4[:], VN[:, 2, 3:4].broadcast_to([P, 4, NSEG]),
            best[:, 3:4].broadcast_to([P, 4, NSEG]), OP.is_gt)
        nc.vector.select(best[:], m4[:], VN[:, 2], best[:])

        # rsqn = 1/|e| ; normalized eigvec in bf16
        rn = pool.tile([P, 1, NSEG], F32, tag="rn")
        nc.vector.reciprocal(rn[:], best[:, 3:4])
        rsqn = pool.tile([P, 1, NSEG], F32, tag="rsqn")
        nc.scalar.activation(rsqn[:], rn[:], AF.Sqrt)
        en = pool.tile([P, 3, NSEG], F32, tag="en")
        nc.vector.tensor_tensor(
            en[:], best[:, 0:3], rsqn[:].broadcast_to([P, 3, NSEG]), OP.mult)
        eb = pool.tile([P, 3, NSEG], BF16, tag="eb")
        nc.vector.tensor_copy(eb[:], en[:])

        # square z on ACT; r2 = sum of squares on GPSIMD
        sqc_i(2)
        for h, eng in ((0, nc.vector), (1, nc.gpsimd)):
            eng.tensor_tensor(r2h[h][:], sqc[0][h][:], sqc[1][h][:], OP.add)
            eng.tensor_tensor(r2h[h][:], r2h[h][:], sqc[2][h][:], OP.add)

        # ---------------- phase 2 tail: per segment-half --------------------
        # h0 chain: front+back on DVE; h1 chain: front/pr/tree on GPSIMD;
        # x0^2 and np0 sqrts on ACT.
        sc = pool.tile([P, 2, HALF], F32, tag="sc")
        uhs, u2s, xqs = [], [], []
        for h in range(2):
            uhs.append(spt.tile([P, V, HALF], BF16, tag="uh", name=f"uh{h}"))
            u2s.append(spt.tile([P, V, HALF], BF16, tag="uh2", name=f"uh2{h}"))
            xqs.append(spt.tile([P, V, HALF], BF16, tag="x0sq",
                                name=f"x0sq{h}"))
        for h, eng in ((0, nc.vector), (1, nc.gpsimd)):
            sl = slice(h * HALF, (h + 1) * HALF)
            # x0 = xc . e (normalized)
            eng.tensor_tensor(x0h[h][:], X[:, :, sl], cbc(eb, 0, h), OP.mult)
            eng.tensor_tensor(uhs[h][:], Y[:, :, sl], cbc(eb, 1, h), OP.mult)
            eng.tensor_tensor(u2s[h][:], Z[:, :, sl], cbc(eb, 2, h), OP.mult)
            eng.tensor_tensor(x0h[h][:], x0h[h][:], uhs[h][:], OP.add)
            eng.tensor_tensor(x0h[h][:], x0h[h][:], u2s[h][:], OP.add)
            # h = r2 - x0^2 ; np0 = sqrt(max(h, 0))
            nc.scalar.activation(xqs[h][:], x0h[h][:], AF.Square)
            nc.vector.tensor_tensor(r2h[h][:], r2h[h][:], xqs[h][:],
                                    OP.subtract)
            nc.vector.tensor_scalar(out=r2h[h][:], in0=r2h[h][:], scalar1=0.0,
                                    scalar2=None, op0=OP.max)
            nc.scalar.activation(r2h[h][:], r2h[h][:], AF.Sqrt)
            # pr = x0 * np0 ; tree-reduce over v -> sc
            eng.tensor_tensor(x0h[h][:], x0h[h][:], r2h[h][:], OP.mult)
            pr = x0h[h]
            w = V
            while w > 8:
                w //= 2
                eng.tensor_tensor(pr[:, 0:w], pr[:, 0:w], pr[:, w:2 * w],
                                  OP.add)
            nc.vector.tensor_reduce(
                sc[:, h], pr[:, 0:8].rearrange("p v s -> p s v"),
                axis=AX.X, op=OP.add)

        # ---------------- finalize: feats [P, NSEG, 16] ----------------
        feats = pool.tile([P, NSEG, 16], F32, tag="feats")
        sgn = pool.tile([P, 1, NSEG], F32, tag="sgn")
        nc.scalar.activation(sgn[:, 0], sc[:].rearrange("p h s -> p (h s)"),
                             AF.Sign)
        fac = pool.tile([P, 1, NSEG], F32, tag="fac")
        nc.vector.tensor_tensor(fac[:], dirwt[:], rsqn[:], OP.mult)
        nc.vector.tensor_tensor(fac[:], fac[:], sgn[:], OP.mult)

        nc.gpsimd.tensor_copy(
            feats[:, :, 0:3].rearrange("p s k -> p k s"), c3[:])
        bb = pool.tile([P, 6, NSEG], F32, tag="bb")
        nc.vector.tensor_tensor(
            bb[:], amat[:], invw3[:].broadcast_to([P, 6, NSEG]), OP.mult)
        for col, row in ((3, 0), (4, 3), (5, 4), (6, 3), (7, 1), (8, 5),
                         (9, 4), (10, 5), (11, 2)):
            nc.gpsimd.tensor_copy(feats[:, :, col], bb[:, row])
        v0t = pool.tile([P, 3, NSEG], F32, tag="v0t")
        nc.vector.tensor_tensor(
            v0t[:], best[:, 0:3], fac[:].broadcast_to([P, 3, NSEG]), OP.mult)
        nc.gpsimd.tensor_copy(
            feats[:, :, 12:15].rearrange("p s k -> p k s"), v0t[:])
        size_t = pool.tile([P, NSEG], F32, tag="size_t")
        nc.gpsimd.memset(size_t[:], float(V))
        nc.gpsimd.tensor_copy(feats[:, :, 15], size_t[:])

        nc.sync.dma_start(feats_d.rearrange("(p s) k -> p s k", p=P), feats[:])

    if not nc.is_finalized():
        nc.finalize()
    return nc


def kernel(data: np.ndarray, clusts: np.ndarray) -> np.ndarray:
    import ml_dtypes
    BF = ml_dtypes.bfloat16

    data = np.asarray(data, dtype=np.float32)
    clusts_np = np.asarray(clusts)
    C, S = clusts_np.shape
    assert (C, S) == (N_CLUSTS, CLUST_SIZE), (C, S)

    vox = data[:, 1:4]
    g = vox[clusts_np.reshape(-1).astype(np.int64)].reshape(C, S, 3)
    g = g.astype(BF)

    if "nc" not in _CACHED:
        _CACHED["nc"] = build_nc()
    nc = _CACHED["nc"]

    in_maps = []
    for c in range(N_CORES):
        sl = slice(c * C_LOC, (c + 1) * C_LOC)
        gc = g[sl]  # [C_LOC, S, 3]
        m = {}
        for i, n in enumerate("xyz"):
            a4 = gc[:, :, i].reshape(P, NSEG, V)
            # cluster-major segment-minor [P, V, NSEG]
            m[n] = np.ascontiguousarray(a4.transpose(0, 2, 1)).reshape(
                P, V * NSEG)
            # voxel-major [V, C_LOC], column s*128+p = cluster p*32+s
            m[n + "t"] = np.ascontiguousarray(
                a4.transpose(2, 1, 0).reshape(V, C_LOC))
        in_maps.append(m)

    kw = {}
    if PROFILE:
        kw = dict(trace=True)
    res = run_bass_kernel_spmd(nc, in_maps, list(range(N_CORES)), **kw)
    if PROFILE:
        global LAST_RESULT
        LAST_RESULT = res
    out = np.concatenate([res.results[c]["feats"] for c in range(N_CORES)],
                         axis=0)
    return out.astype(np.float32)


# revision 29
# speedup vs baseline: 1.0119x; 1.0004x over previous
"""Trainium2 Bass kernel: per-cluster PCA geometry features (segment reduce).

Problem: data [4194304, 6] f32, clusts [32768, 128] int — per cluster of 128
voxels compute: center (mean of xyz), normalized covariance B = A/lmax,
principal axis v0 scaled by dirwt = 1 - lmid/lmax with a sign fix, size.

v2 design (cost-model driven):
- 32768 clusters sharded over 8 cores (4096 each); cluster c = p*32 + s
  (partition p, segment s). All plane data in bf16.
- Dual layout per coordinate: voxel-major [V, C_LOC] for PE column sums
  (phase-1 moments), cluster-major *segment-minor* [P, V, S] so per-cluster
  scalar broadcasts keep a packed (stride-1) last dim -> DVE 2x mode.
- Phase 1: squares on ACT, cross products on GPSIMD, sums via 288 tiny
  PE matmuls into one PSUM tile.
- Eigensolve: batched small-tile ops on [128, K, 32] tiles (trig method),
  eigenvector via row-cross-products on a doubled-rows tile so the three
  candidate cross products are affine-sliced batched ops.
- Phase 2 (per segment-half, pipelined): xc = X - c, r2 = |xc|^2 via ACT
  squares + GPSIMD adds, x0' = xc . e (unnormalized eigvec, avoids a sqrt
  on the critical path), h = r2 - x0'^2/|e|^2, np0 = ACT sqrt of clamped h,
  sign criterion sc = sum_v x0'*np0 via packed tree-reduction.
"""
import numpy as np
from contextlib import ExitStack

import concourse.bass as bass
import concourse.bacc as bacc
import concourse.tile as tile
from concourse import mybir
from concourse.bass_utils import run_bass_kernel_spmd

N_CLUSTS = 32768
CLUST_SIZE = 128
N_CORES = 8
C_LOC = N_CLUSTS // N_CORES   # 4096 clusters per core
P = 128                       # SBUF partitions
NSEG = C_LOC // P             # 32 clusters (segments) per partition
V = CLUST_SIZE                # 128 voxels per cluster
HALF = NSEG // 2

F32 = mybir.dt.float32
BF16 = mybir.dt.bfloat16
AF = mybir.ActivationFunctionType
OP = mybir.AluOpType
AX = mybir.AxisListType

_CACHED = {}
PROFILE = False          # set by test.py to capture an NTFF trace
LAST_RESULT = None       # BassKernelResults of the last run when PROFILE
INPUT_SHAPES = {
    "x": (P, V * NSEG), "y": (P, V * NSEG), "z": (P, V * NSEG),
    "xt": (V, C_LOC), "yt": (V, C_LOC), "zt": (V, C_LOC),
}
INPUT_DTYPE = "bfloat16"


def build_nc():
    nc = bacc.Bacc()
    # cluster-major segment-minor: row p holds [v, s] (v outer, s inner)
    x_d = nc.dram_tensor("x", [P, V * NSEG], BF16, kind="ExternalInput").ap()
    y_d = nc.dram_tensor("y", [P, V * NSEG], BF16, kind="ExternalInput").ap()
    z_d = nc.dram_tensor("z", [P, V * NSEG], BF16, kind="ExternalInput").ap()
    # voxel-major: column s*128+p holds cluster p*32+s
    xt_d = nc.dram_tensor("xt", [V, C_LOC], BF16, kind="ExternalInput").ap()
    yt_d = nc.dram_tensor("yt", [V, C_LOC], BF16, kind="ExternalInput").ap()
    zt_d = nc.dram_tensor("zt", [V, C_LOC], BF16, kind="ExternalInput").ap()
    feats_d = nc.dram_tensor("feats", [C_LOC, 16], F32, kind="ExternalOutput").ap()

    with tile.TileContext(nc) as tc, ExitStack() as ctx:
        pool = ctx.enter_context(tc.tile_pool(name="main", bufs=1))
        sp = ctx.enter_context(tc.tile_pool(name="scratch", bufs=6))
        spc = ctx.enter_context(tc.tile_pool(name="scratchc", bufs=4))
        spt = ctx.enter_context(tc.tile_pool(name="scratcht", bufs=3))
        pp = ctx.enter_context(tc.tile_pool(name="psum", bufs=1, space="PSUM"))

        ones = pool.tile([P, 1], BF16, tag="ones")
        nc.gpsimd.memset(ones[:], 1.0)
        warm = pool.tile([P, 1], F32, tag="warm")
        nc.gpsimd.memset(warm[:], 1.0)
        nc.scalar.activation(warm[:], warm[:], AF.Sqrt)

        # ---------------- input DMAs ----------------
        Xt = pool.tile([P, C_LOC], BF16, tag="Xt")
        Yt = pool.tile([P, C_LOC], BF16, tag="Yt")
        Zt = pool.tile([P, C_LOC], BF16, tag="Zt")
        X = pool.tile([P, V, NSEG], BF16, tag="X")
        Y = pool.tile([P, V, NSEG], BF16, tag="Y")
        Z = pool.tile([P, V, NSEG], BF16, tag="Z")
        nc.sync.dma_start(Xt[:], xt_d)
        nc.sync.dma_start(Yt[:], yt_d)
        nc.sync.dma_start(Zt[:], zt_d)
        nc.sync.dma_start(X[:], x_d.rearrange("p (v s) -> p v s", s=NSEG))
        nc.sync.dma_start(Y[:], y_d.rearrange("p (v s) -> p v s", s=NSEG))
        nc.sync.dma_start(Z[:], z_d.rearrange("p (v s) -> p v s", s=NSEG))

        # ---------------- phase 1: moments via PE column sums ----------------
        psum = pp.tile([P, 9, NSEG], F32, tag="psums")

        def pe_colsums(plane_t, k):
            for j in range(NSEG):
                nc.tensor.matmul(
                    out=psum[:, k, j : j + 1],
                    lhsT=plane_t[:, j * P : (j + 1) * P],
                    rhs=ones[:, 0:1],
                    start=True,
                    stop=True,
                )

        pe_colsums(Xt, 0)
        pe_colsums(Yt, 1)
        pe_colsums(Zt, 2)
        mom = pool.tile([P, 9, NSEG], F32, tag="mom")
        nc.vector.tensor_copy(mom[:, 0:3], psum[:, 0:3])
        inv_s = 1.0 / V
        c3 = pool.tile([P, 3, NSEG], F32, tag="c3")
        nc.vector.tensor_scalar(out=c3[:], in0=mom[:, 0:3], scalar1=inv_s,
                                scalar2=None, op0=OP.mult)
        cb = pool.tile([P, 3, NSEG], BF16, tag="cb")
        nc.vector.tensor_copy(cb[:], c3[:])
        sq1 = sp.tile([P, C_LOC], BF16, tag="vplane", name="sq1")
        nc.scalar.activation(sq1[:], Xt[:], AF.Square)
        pe_colsums(sq1, 3)
        sq2 = sp.tile([P, C_LOC], BF16, tag="vplane", name="sq2")
        nc.scalar.activation(sq2[:], Yt[:], AF.Square)
        pe_colsums(sq2, 4)
        pr1 = sp.tile([P, C_LOC], BF16, tag="vplane", name="pr1")
        nc.gpsimd.tensor_tensor(pr1[:], Xt[:], Yt[:], OP.mult)
        pe_colsums(pr1, 6)
        pr2 = sp.tile([P, C_LOC], BF16, tag="vplane", name="pr2")
        nc.vector.tensor_tensor(pr2[:], Xt[:], Zt[:], OP.mult)
        pe_colsums(pr2, 7)
        # last plane's square split across ACT and DVE halves
        sq3 = sp.tile([P, C_LOC], BF16, tag="vplane", name="sq3")
        nc.scalar.activation(sq3[:, 0:C_LOC // 2], Zt[:, 0:C_LOC // 2],
                             AF.Square)
        nc.vector.tensor_tensor(sq3[:, C_LOC // 2:], Zt[:, C_LOC // 2:],
                                Zt[:, C_LOC // 2:], OP.mult)
        pe_colsums(sq3, 5)
        pr3 = sp.tile([P, C_LOC], BF16, tag="vplane", name="pr3")
        nc.gpsimd.tensor_tensor(pr3[:], Yt[:], Zt[:], OP.mult)
        pe_colsums(pr3, 8)

        # second moments: rows 3-8 = [Mxx Myy Mzz Mxy Mxz Myz]
        nc.vector.tensor_copy(mom[:, 3:9], psum[:, 3:9])

        def bcast_v(t, lo, hi, sl, n=1):
            # [P, K, NSEG] rows lo:hi, segments sl -> [P, n, V, W] broadcast
            w = sl.stop - sl.start
            if n == 1:
                return t[:, lo:hi, sl].broadcast_to([P, V, w]) if hi - lo == 1 \
                    else None
            return None

        # a = M - S*c ; amat rows: [axx, ayy, azz, axy, axz, ayz]
        amat = pool.tile([P, 6, NSEG], F32, tag="amat")
        t6 = pool.tile([P, 6, NSEG], F32, tag="t6")
        nc.vector.tensor_tensor(t6[:, 0:3], mom[:, 0:3], c3[:], OP.mult)
        nc.vector.tensor_tensor(
            t6[:, 3:5], mom[:, 0:1].broadcast_to([P, 2, NSEG]),
            c3[:, 1:3], OP.mult)
        nc.vector.tensor_tensor(t6[:, 5:6], mom[:, 1:2], c3[:, 2:3], OP.mult)
        nc.vector.tensor_tensor(amat[:], mom[:, 3:9], t6[:], OP.subtract)

        # ---- phase 2 state ------------------------------------------------
        planes = [X, Y, Z]
        r2h = [pool.tile([P, V, HALF], BF16, tag=f"r2h{h}", name=f"r2h{h}")
               for h in range(2)]
        x0h = [pool.tile([P, V, HALF], BF16, tag=f"x0h{h}", name=f"x0h{h}")
               for h in range(2)]
        sqc = [[spc.tile([P, V, HALF], BF16, tag="sqc", name=f"sqc{i}{h}")
                for h in range(2)] for i in range(3)]

        def cbc(t, row, h, w=HALF):
            # [P, K, NSEG] row -> [P, V, w] broadcast over voxels
            return t[:, row:row + 1, h * HALF:h * HALF + w].broadcast_to(
                [P, V, w])

        def xc_i(i, eng=None):
            # center one coordinate (both halves)
            eng = eng or (nc.vector if i == 0 else nc.gpsimd)
            pl = planes[i]
            for h in range(2):
                sl = slice(h * HALF, (h + 1) * HALF)
                eng.tensor_tensor(pl[:, :, sl], pl[:, :, sl],
                                  cbc(cb, i, h), OP.subtract)

        def sqc_i(i):
            # square one centered coordinate (both halves); z on Pool right
            # after its centering (no hop, unclogs ACT), x/y on ACT
            pl = planes[i]
            for h in range(2):
                sl = slice(h * HALF, (h + 1) * HALF)
                if i == 2:
                    nc.gpsimd.tensor_tensor(sqc[i][h][:], pl[:, :, sl],
                                            pl[:, :, sl], OP.mult)
                else:
                    nc.scalar.activation(sqc[i][h][:], pl[:, :, sl], AF.Square)

        # a = M - S*c ; amat rows: [axx, ayy, azz, axy, axz, ayz]
        amat = pool.tile([P, 6, NSEG], F32, tag="amat")
        t6 = pool.tile([P, 6, NSEG], F32, tag="t6")
        nc.vector.tensor_tensor(t6[:, 0:3], mom[:, 0:3], c3[:], OP.mult)
        nc.vector.tensor_tensor(
            t6[:, 3:5], mom[:, 0:1].broadcast_to([P, 2, NSEG]),
            c3[:, 1:3], OP.mult)
        nc.vector.tensor_tensor(t6[:, 5:6], mom[:, 1:2], c3[:, 2:3], OP.mult)
        nc.vector.tensor_tensor(amat[:], mom[:, 3:9], t6[:], OP.subtract)

        # ---------------- eigensolve (batched [P, K, NSEG] f32) -------------
        q = pool.tile([P, 1, NSEG], F32, tag="q")
        t1a = pool.tile([P, 1, NSEG], F32, tag="t1a")
        t1b = pool.tile([P, 1, NSEG], F32, tag="t1b")
        nc.vector.tensor_tensor(t1a[:], amat[:, 0:1], amat[:, 1:2], OP.add)
        nc.vector.tensor_tensor(t1a[:], t1a[:], amat[:, 2:3], OP.add)
        nc.vector.tensor_scalar(out=q[:], in0=t1a[:], scalar1=1.0 / 3.0,
                                scalar2=None, op0=OP.mult)
        # ba rows: [b11, b22, b33, axy, axz, ayz]  (traceless part)
        ba = pool.tile([P, 6, NSEG], F32, tag="ba")
        nc.vector.tensor_tensor(
            ba[:, 0:3], amat[:, 0:3], q[:].broadcast_to([P, 3, NSEG]),
            OP.subtract)
        nc.vector.tensor_copy(ba[:, 3:6], amat[:, 3:6])

        # p2 = sum(bd^2) + 2*sum(off^2) ; p = sqrt(p2/6); invp = 1/p
        sq6 = pool.tile([P, 6, NSEG], F32, tag="sq6")
        nc.vector.tensor_tensor(sq6[:], ba[:], ba[:], OP.mult)
        w3t = pool.tile([P, 3, NSEG], F32, tag="w3t")
        nc.vector.scalar_tensor_tensor(out=w3t[:], in0=sq6[:, 3:6], scalar=2.0,
                                       in1=sq6[:, 0:3], op0=OP.mult, op1=OP.add)
        p2s = pool.tile([P, 1, NSEG], F32, tag="p2s")
        nc.vector.tensor_tensor(t1a[:], w3t[:, 0:1], w3t[:, 1:2], OP.add)
        nc.vector.tensor_tensor(p2s[:], t1a[:], w3t[:, 2:3], OP.add)
        p_t = pool.tile([P, 1, NSEG], F32, tag="p_t")
        nc.scalar.activation(p_t[:], p2s[:], AF.Sqrt, scale=1.0 / 6.0)
        invp = pool.tile([P, 1, NSEG], F32, tag="invp")
        nc.vector.reciprocal(invp[:], p_t[:])

        # center + square x (DVE was busy with the eigen chain until now)
        xc_i(0)
        sqc_i(0)
        xc_i(1)

        # det of the raw traceless matrix ba on DVE (overlaps the ACT sqrt
        # for p); r = det(ba) * invp^3 / 2, clamped to [-1, 1].
        # minors vs row1: m1 = b22*b33 - b23^2 ; m2 = b12*b33 - b23*b13 ;
        # m3 = b12*b23 - b22*b13 ; det = b11*m1 - b12*m2 + b13*m3
        r_t = pool.tile([P, 1, NSEG], F32, tag="r_t")
        det_a = pool.tile([P, 3, NSEG], F32, tag="det_a")
        det_b = pool.tile([P, 3, NSEG], F32, tag="det_b")
        nc.vector.tensor_tensor(det_a[:, 0:1], ba[:, 1:2], ba[:, 2:3],
                                OP.mult)
        nc.vector.tensor_tensor(
            det_a[:, 1:3], ba[:, 3:4].broadcast_to([P, 2, NSEG]),
            ba[:, 2:6:3], OP.mult)
        nc.vector.tensor_tensor(det_b[:, 0:1], ba[:, 5:6], ba[:, 5:6],
                                OP.mult)
        nc.vector.tensor_tensor(det_b[:, 1:2], ba[:, 5:6], ba[:, 4:5],
                                OP.mult)
        nc.vector.tensor_tensor(det_b[:, 2:3], ba[:, 1:2], ba[:, 4:5],
                                OP.mult)
        nc.vector.tensor_tensor(det_a[:], det_a[:], det_b[:], OP.subtract)
        nc.vector.tensor_tensor(det_b[:, 0:1], ba[:, 0:1], det_a[:, 0:1],
                                OP.mult)
        nc.vector.tensor_tensor(det_b[:, 1:3], ba[:, 3:5], det_a[:, 1:3],
                                OP.mult)
        nc.vector.tensor_tensor(t1b[:], det_b[:, 0:1], det_b[:, 1:2],
                                OP.subtract)
        nc.vector.tensor_tensor(t1b[:], t1b[:], det_b[:, 2:3], OP.add)
        # r = det * invp^3 / 2, clamped
        nc.vector.tensor_tensor(t1a[:], invp[:], invp[:], OP.mult)
        nc.vector.tensor_tensor(t1a[:], t1a[:], invp[:], OP.mult)
        nc.vector.tensor_tensor(t1b[:], t1b[:], t1a[:], OP.mult)
        nc.vector.tensor_scalar(out=r_t[:], in0=t1b[:], scalar1=0.5,
                                scalar2=1.0, op0=OP.mult, op1=OP.min)
        nc.vector.tensor_scalar(out=r_t[:], in0=r_t[:], scalar1=-1.0,
                                scalar2=None, op0=OP.max)

        # square y on ACT; center z on GPSIMD (z DMA lands about now)
        sqc_i(1)
        xc_i(2)

        # f = cos(acos(r)/3): largest root of 4f^3 - 3f = r.
        # Chebyshev deg-8 seed (err 2.2e-2) + 2 Newton steps -> 5.5e-3 max
        # (only near r=-1, where the top eigenpair degenerates and dirwt -> 0).
        SEED = [0.8649279174994734, 0.15532929881670984, -0.0021054445875550026,
                0.14553392157332898, -0.3069890177054866, -0.3028838631742596,
                0.5725439670593226, 0.24058719928428143, -0.3682048402442527]
        f_t = pool.tile([P, 1, NSEG], F32, tag="f_t")
        nc.vector.tensor_scalar(out=f_t[:], in0=r_t[:], scalar1=0.0,
                                scalar2=SEED[8], op0=OP.mult, op1=OP.add)
        for k in range(8, 0, -1):
            # f <- (f + a_k) * r, then finally + a_0
            nc.vector.scalar_tensor_tensor(
                out=f_t[:], in0=f_t[:], scalar=0.0 if k == 8 else SEED[k],
                in1=r_t[:], op0=OP.add, op1=OP.mult)
        nc.vector.tensor_scalar(out=f_t[:], in0=f_t[:], scalar1=SEED[0],
                                scalar2=None, op0=OP.add)
        nc.vector.tensor_scalar(out=f_t[:], in0=f_t[:], scalar1=0.5,
                                scalar2=1.0, op0=OP.max, op1=OP.min)
        f2 = pool.tile([P, 1, NSEG], F32, tag="f2")
        for _ in range(1):
            nc.vector.tensor_tensor(f2[:], f_t[:], f_t[:], OP.mult)
            nc.vector.tensor_scalar(out=t1a[:], in0=f2[:], scalar1=12.0,
                                    scalar2=-3.0, op0=OP.mult, op1=OP.add)
            nc.vector.tensor_scalar(out=t1a[:], in0=t1a[:], scalar1=1e-3,
                                    scalar2=None, op0=OP.max)
            nc.vector.reciprocal(t1a[:], t1a[:])
            nc.vector.tensor_scalar(out=t1b[:], in0=f2[:], scalar1=4.0,
                                    scalar2=-3.0, op0=OP.mult, op1=OP.add)
            nc.vector.tensor_tensor(t1b[:], t1b[:], f_t[:], OP.mult)
            nc.vector.tensor_tensor(t1b[:], t1b[:], r_t[:], OP.subtract)
            nc.vector.tensor_tensor(t1b[:], t1b[:], t1a[:], OP.mult)
            nc.vector.tensor_tensor(f_t[:], f_t[:], t1b[:], OP.subtract)
            nc.vector.tensor_scalar(out=f_t[:], in0=f_t[:], scalar1=0.5,
                                    scalar2=1.0, op0=OP.max, op1=OP.min)

        # w3 = q + 2*p*f ; w2 = q + (sqrt(2*p2*(1-f^2)) - 2*p*f)/2
        # (w2 from the deflated quadratic t^2 + t3 t + t3^2 - p2/2 = 0)
        w3 = pool.tile([P, 1, NSEG], F32, tag="w3")
        w2 = pool.tile([P, 1, NSEG], F32, tag="w2")
        m1 = pool.tile([P, 1, NSEG], F32, tag="m1")
        nc.vector.tensor_tensor(m1[:], p_t[:], f_t[:], OP.mult)   # p*f
        nc.vector.scalar_tensor_tensor(out=w3[:], in0=m1[:], scalar=2.0,
                                       in1=q[:], op0=OP.mult, op1=OP.add)
        nc.vector.tensor_tensor(f2[:], f_t[:], f_t[:], OP.mult)
        nc.vector.tensor_scalar(out=t1b[:], in0=f2[:], scalar1=-1.0,
                                scalar2=1.0, op0=OP.mult, op1=OP.add)
        nc.vector.scalar_tensor_tensor(out=t1b[:], in0=p2s[:], scalar=2.0,
                                       in1=t1b[:], op0=OP.mult, op1=OP.mult)
        nc.scalar.activation(t1a[:], t1b[:], AF.Sqrt)   # sqrt(disc)
        nc.vector.scalar_tensor_tensor(out=t1b[:], in0=t1a[:], scalar=0.5,
                                       in1=m1[:], op0=OP.mult, op1=OP.subtract)
        nc.vector.tensor_tensor(w2[:], t1b[:], q[:], OP.add)

        invw3 = pool.tile([P, 1, NSEG], F32, tag="invw3")
        nc.vector.reciprocal(invw3[:], w3[:])
        dirwt = pool.tile([P, 1, NSEG], F32, tag="dirwt")
        nc.vector.tensor_tensor(dirwt[:], w2[:], invw3[:], OP.mult)
        nc.vector.tensor_scalar(out=dirwt[:], in0=dirwt[:], scalar1=-1.0,
                                scalar2=1.0, op0=OP.mult, op1=OP.add)

        # ---- eigenvector: batched row cross products -------------------
        # rows of (A - w3 I): r1 = (d1, axy, axz), r2 = (axy, d2, ayz),
        # r3 = (axz, ayz, d3); D = per-vector doubled rows.
        rows = pool.tile([P, 9, NSEG], F32, tag="rows")
        nc.vector.tensor_tensor(
            rows[:, 0:9:4], amat[:, 0:3],
            w3[:].broadcast_to([P, 3, NSEG]), OP.subtract)
        nc.vector.tensor_copy(rows[:, 1:2], amat[:, 3:4])   # axy
        nc.vector.tensor_copy(rows[:, 2:3], amat[:, 4:5])   # axz
        nc.vector.tensor_copy(rows[:, 3:4], amat[:, 3:4])   # axy
        nc.vector.tensor_copy(rows[:, 5:6], amat[:, 5:6])   # ayz
        nc.vector.tensor_copy(rows[:, 6:7], amat[:, 4:5])   # axz
        nc.vector.tensor_copy(rows[:, 7:8], amat[:, 5:6])   # ayz
        D = pool.tile([P, 3, 2, 3, NSEG], F32, tag="D")
        rows_g = rows[:].rearrange("p (g c) s -> p g c s", g=3)
        nc.vector.tensor_copy(D[:, :, 0], rows_g)
        nc.vector.tensor_copy(D[:, :, 1], rows_g)
        Dg = D[:].rearrange("p g r c s -> p g (r c) s")  # [P, 3, 6, NSEG]

        # VN rows: cand_a(0:3), na(3), cand_b(4:7), nb(7), cand_c(8:11), nc(11)
        VN = pool.tile([P, 3, 4, NSEG], F32, tag="VN")
        ta = pool.tile([P, 2, 3, NSEG], F32, tag="ta")
        tb = pool.tile([P, 2, 3, NSEG], F32, tag="tb")
        r1s1 = Dg[:, 0:1, 1:4].broadcast_to([P, 2, 3, NSEG])
        r1s2 = Dg[:, 0:1, 2:5].broadcast_to([P, 2, 3, NSEG])
        nc.vector.tensor_tensor(ta[:], r1s1, Dg[:, 1:3, 2:5], OP.mult)
        nc.vector.tensor_tensor(tb[:], r1s2, Dg[:, 1:3, 1:4], OP.mult)
        nc.vector.tensor_tensor(ta[:], ta[:], tb[:], OP.subtract)
        nc.vector.tensor_copy(VN[:, 0:2, 0:3], ta[:])
        nc.vector.tensor_tensor(tb[:, 0], Dg[:, 1, 1:4], Dg[:, 2, 2:5], OP.mult)
        nc.vector.tensor_tensor(tb[:, 1], Dg[:, 1, 2:5], Dg[:, 2, 1:4], OP.mult)
        nc.vector.tensor_tensor(VN[:, 2, 0:3], tb[:, 0], tb[:, 1], OP.subtract)

        # norms into VN[:, :, 3]
        sq9 = pool.tile([P, 3, 3, NSEG], F32, tag="sq9")
        nc.vector.tensor_tensor(sq9[:], VN[:, :, 0:3], VN[:, :, 0:3], OP.mult)
        nc.vector.tensor_tensor(ta[:, 0], sq9[:, :, 0], sq9[:, :, 1], OP.add)
        nc.vector.tensor_tensor(VN[:, :, 3], ta[:, 0], sq9[:, :, 2], OP.add)

        # select the largest-norm candidate (two rounds, batched 4-row select)
        m4 = pool.tile([P, 4, NSEG], mybir.dt.uint8, tag="mask4")
        best = pool.tile([P, 4, NSEG], F32, tag="best")
        nc.vector.tensor_tensor(
            m4[:], VN[:, 1, 3:4].broadcast_to([P, 4, NSEG]),
            VN[:, 0, 3:4].broadcast_to([P, 4, NSEG]), OP.is_gt)
        nc.vector.select(best[:], m4[:], VN[:, 1], VN[:, 0])
        nc.vector.tensor_tensor(
            m4[:], VN[:, 2, 3:4].broadcast_to([P, 4, NSEG]),
            best[:, 3:4].broadcast_to([P, 4, NSEG]), OP.is_gt)
        nc.vector.select(best[:], m4[:], VN[:, 2], best[:])

        # rsqn = 1/|e| ; normalized eigvec in bf16
        rn = pool.tile([P, 1, NSEG], F32, tag="rn")
        nc.vector.reciprocal(rn[:], best[:, 3:4])
        rsqn = pool.tile([P, 1, NSEG], F32, tag="rsqn")
        nc.scalar.activation(rsqn[:], rn[:], AF.Sqrt)
        en = pool.tile([P, 3, NSEG], F32, tag="en")
        nc.vector.tensor_tensor(
            en[:], best[:, 0:3], rsqn[:].broadcast_to([P, 3, NSEG]), OP.mult)
        eb = pool.tile([P, 3, NSEG], BF16, tag="eb")
        nc.vector.tensor_copy(eb[:], en[:])

        # square z on ACT; r2 = sum of squares on GPSIMD
        sqc_i(2)
        for h, eng in ((0, nc.vector), (1, nc.gpsimd)):
            eng.tensor_tensor(r2h[h][:], sqc[0][h][:], sqc[1][h][:], OP.add)
            eng.tensor_tensor(r2h[h][:], r2h[h][:], sqc[2][h][:], OP.add)

        # ---------------- phase 2 tail: per segment-half --------------------
        # h0 chain: front+back on DVE; h1 chain: front/pr/tree on GPSIMD;
        # x0^2 and np0 sqrts on ACT.
        sc = pool.tile([P, 2, HALF], F32, tag="sc")
        uhs, u2s, xqs = [], [], []
        for h in range(2):
            uhs.append(spt.tile([P, V, HALF], BF16, tag="uh", name=f"uh{h}"))
            u2s.append(spt.tile([P, V, HALF], BF16, tag="uh2", name=f"uh2{h}"))
            xqs.append(spt.tile([P, V, HALF], BF16, tag="x0sq",
                                name=f"x0sq{h}"))
        for h, eng in ((0, nc.vector), (1, nc.gpsimd)):
            sl = slice(h * HALF, (h + 1) * HALF)
            # x0 = xc . e (normalized)
            eng.tensor_tensor(x0h[h][:], X[:, :, sl], cbc(eb, 0, h), OP.mult)
            eng.tensor_tensor(uhs[h][:], Y[:, :, sl], cbc(eb, 1, h), OP.mult)
            eng.tensor_tensor(u2s[h][:], Z[:, :, sl], cbc(eb, 2, h), OP.mult)
            eng.tensor_tensor(x0h[h][:], x0h[h][:], uhs[h][:], OP.add)
            eng.tensor_tensor(x0h[h][:], x0h[h][:], u2s[h][:], OP.add)
            # h = r2 - x0^2 ; np0 = sqrt(max(h, 0))
            nc.scalar.activation(xqs[h][:], x0h[h][:], AF.Square)
            nc.vector.tensor_tensor(r2h[h][:], r2h[h][:], xqs[h][:],
                                    OP.subtract)
            nc.vector.tensor_scalar(out=r2h[h][:], in0=r2h[h][:], scalar1=0.0,
                                    scalar2=None, op0=OP.max)
            nc.scalar.activation(r2h[h][:], r2h[h][:], AF.Sqrt)
            # pr = x0 * np0 ; tree-reduce over v -> sc
            eng.tensor_tensor(x0h[h][:], x0h[h][:], r2h[h][:], OP.mult)
            pr = x0h[h]
            w = V
            while w > 8:
                w //= 2
                eng.tensor_tensor(pr[:, 0:w], pr[:, 0:w], pr[:, w:2 * w],
                                  OP.add)
            nc.vector.tensor_reduce(
                sc[:, h], pr[:, 0:8].rearrange("p v s -> p s v"),
                axis=AX.X, op=OP.add)

        # ---------------- finalize: feats [P, NSEG, 16] ----------------
        feats = pool.tile([P, NSEG, 16], F32, tag="feats")
        sgn = pool.tile([P, 1, NSEG], F32, tag="sgn")
        nc.scalar.activation(sgn[:, 0], sc[:].rearrange("p h s -> p (h s)"),
                             AF.Sign)
        fac = pool.tile([P, 1, NSEG], F32, tag="fac")
        nc.vector.tensor_tensor(fac[:], dirwt[:], rsqn[:], OP.mult)
        nc.vector.tensor_tensor(fac[:], fac[:], sgn[:], OP.mult)

        nc.gpsimd.tensor_copy(
            feats[:, :, 0:3].rearrange("p s k -> p k s"), c3[:])
        bb = pool.tile([P, 6, NSEG], F32, tag="bb")
        nc.vector.tensor_tensor(
            bb[:], amat[:], invw3[:].broadcast_to([P, 6, NSEG]), OP.mult)
        for col, row in ((3, 0), (4, 3), (5, 4), (6, 3), (7, 1), (8, 5),
                         (9, 4), (10, 5), (11, 2)):
            nc.gpsimd.tensor_copy(feats[:, :, col], bb[:, row])
        v0t = pool.tile([P, 3, NSEG], F32, tag="v0t")
        nc.vector.tensor_tensor(
            v0t[:], best[:, 0:3], fac[:].broadcast_to([P, 3, NSEG]), OP.mult)
        nc.gpsimd.tensor_copy(
            feats[:, :, 12:15].rearrange("p s k -> p k s"), v0t[:])
        size_t = pool.tile([P, NSEG], F32, tag="size_t")
        nc.gpsimd.memset(size_t[:], float(V))
        nc.gpsimd.tensor_copy(feats[:, :, 15], size_t[:])

        nc.sync.dma_start(feats_d.rearrange("(p s) k -> p s k", p=P), feats[:])

    if not nc.is_finalized():
        nc.finalize()
    return nc


def kernel(data: np.ndarray, clusts: np.ndarray) -> np.ndarray:
    import ml_dtypes
    BF = ml_dtypes.bfloat16

    data = np.asarray(data, dtype=np.float32)
    clusts_np = np.asarray(clusts)
    C, S = clusts_np.shape
    assert (C, S) == (N_CLUSTS, CLUST_SIZE), (C, S)

    vox = data[:, 1:4]
    g = vox[clusts_np.reshape(-1).astype(np.int64)].reshape(C, S, 3)
    g = g.astype(BF)

    if "nc" not in _CACHED:
        _CACHED["nc"] = build_nc()
    nc = _CACHED["nc"]

    in_maps = []
    for c in range(N_CORES):
        sl = slice(c * C_LOC, (c + 1) * C_LOC)
        gc = g[sl]  # [C_LOC, S, 3]
        m = {}
        for i, n in enumerate("xyz"):
            a4 = gc[:, :, i].reshape(P, NSEG, V)
            # cluster-major segment-minor [P, V, NSEG]
            m[n] = np.ascontiguousarray(a4.transpose(0, 2, 1)).reshape(
                P, V * NSEG)
            # voxel-major [V, C_LOC], column s*128+p = cluster p*32+s
            m[n + "t"] = np.ascontiguousarray(
                a4.transpose(2, 1, 0).reshape(V, C_LOC))
        in_maps.append(m)

    kw = {}
    if PROFILE:
        kw = dict(trace=True)
    res = run_bass_kernel_spmd(nc, in_maps, list(range(N_CORES)), **kw)
    if PROFILE:
        global LAST_RESULT
        LAST_RESULT = res
    out = np.concatenate([res.results[c]["feats"] for c in range(N_CORES)],
                         axis=0)
    return out.astype(np.float32)
